# revision 19
# baseline (speedup 1.0000x reference)
"""MoE feed-forward (B=4,S=2048,D=1024,F=2048,E=8,top-2) on 8 trn2 NeuronCores.

Strategy (expert-parallel + balanced tail):
 - Host computes the top-2 softmax routing and dispatches tokens.
 - Core e's "main" blocks hold up to 2048 tokens of expert e (4 blocks of
   512, transposed to [D, 2048] fp16, zero padded).
 - Tokens beyond 2048 of overloaded experts are split into parts of <= R
   and placed in a per-core "tail" block of width R whose weight set is an
   independent kernel input -- so each core processes 2048+R columns
   instead of C_max, balancing the matmul stream across cores.
 - Device per core: h1 = W1^T x, s = silu(h1), h3 = W3^T x, g = s*h3,
   y^T = (W2^T g) * w, streamed over column blocks; all matmuls
   fp16 x fp16 -> fp32 PSUM.  Tail matmuls ride along the last main
   block's (f,k) loops; tail weights stream through small SBUF windows
   with DMA emission interleaved into the loops (just-in-time, so the
   DMA queue never blocks on window reuse).
 - A memset + 8 garbage matmuls at t=0 warm the PE HAM clock gate while
   the first real DMAs land, so real matmuls start at 2.4 GHz.
"""

import numpy as np

import concourse.bass as bass
import concourse.tile as tile
from concourse import bacc, mybir
from concourse.bass_utils import run_bass_kernel_spmd

B, S, D, F, E, TOPK = 4, 2048, 1024, 2048, 8, 2
N_CORES = 8
KD = D // 128   # 8 contraction tiles for D
KF = F // 128   # 16 contraction tiles for F
MAIN = 2048     # main capacity per core

_nc_cache = {}


def _blocks_for(C):
    """Split C columns into blocks of <=512, all >=~251 where possible."""
    if C <= 512:
        return [C]
    n = -(-C // 512)
    r = C - 512 * (n - 2)
    a = (r + 1) // 2
    return [512] * (n - 2) + [a, r - a]


def _build_nc(C, R):
    """Per-core Bass program: main capacity C (blocks of <=512) plus an
    optional tail block of width R with its own weight set."""
    f16 = mybir.dt.float16
    f32 = mybir.dt.float32
    blocks = _blocks_for(C)
    last = len(blocks) - 1

    nc = bacc.Bacc(None, target_bir_lowering=False, enable_partition_id=False,
                   monotonic_sem_count=0)
    xT = nc.dram_tensor("xT", [D, C], f16, kind="ExternalInput")
    wt = nc.dram_tensor("wt", [1, C], f32, kind="ExternalInput")
    W1 = nc.dram_tensor("W1", [D, F], f16, kind="ExternalInput")
    W3 = nc.dram_tensor("W3", [D, F], f16, kind="ExternalInput")
    W2 = nc.dram_tensor("W2", [F, D], f16, kind="ExternalInput")
    yT = nc.dram_tensor("yT", [D, C], f16, kind="ExternalOutput")
    if R:
        xTt = nc.dram_tensor("xTt", [D, R], f16, kind="ExternalInput")
        wtt = nc.dram_tensor("wtt", [1, R], f32, kind="ExternalInput")
        W1t = nc.dram_tensor("W1t", [D, F], f16, kind="ExternalInput")
        W3t = nc.dram_tensor("W3t", [D, F], f16, kind="ExternalInput")
        W2t = nc.dram_tensor("W2t", [F, D], f16, kind="ExternalInput")
        yTt = nc.dram_tensor("yTt", [D, R], f16, kind="ExternalOutput")

    # [D, nb] slab viewed as [128, KD, nb] (partition-major tiles)
    def slab(t, c0, nb):
        return t[:, c0:c0 + nb].rearrange("(k p) n -> p k n", p=128)

    def bcast(t, c0, nb):
        return bass.AP(tensor=t.ap().tensor, offset=c0, ap=[[0, 128], [1, nb]])

    with tile.TileContext(nc) as tc:
        with (
            tc.tile_pool(name="wpool", bufs=1) as wpool,
            tc.tile_pool(name="xpool", bufs=2) as xpool,
            tc.tile_pool(name="gpool", bufs=1) as gpool,
            tc.tile_pool(name="spool", bufs=1) as spool,
            tc.tile_pool(name="ypool", bufs=1) as ypool,
            tc.tile_pool(name="wbpool", bufs=2) as wbpool,
            tc.tile_pool(name="twpool", bufs=5) as twpool,
            tc.tile_pool(name="w2tpool", bufs=5) as w2tpool,
            tc.tile_pool(name="tspool", bufs=1) as tspool,
            tc.tile_pool(name="psA", bufs=3, space="PSUM") as psA,
            tc.tile_pool(name="psB", bufs=2, space="PSUM") as psB,
            tc.tile_pool(name="psY", bufs=2, space="PSUM") as psY,
            tc.tile_pool(name="psT", bufs=1, space="PSUM") as psT,
        ):
            # --- HAM warmup: PE busy on garbage while first DMAs land ---
            dum = spool.tile([128, 256], f16, tag="dum")
            nc.vector.memset(dum[:, :], 0.0)
            psd = psA.tile([128, 512], f32, tag="ps1", name="psdum")
            for _ in range(16):
                nc.tensor.matmul(psd[:, 0:256], lhsT=dum[:, 0:128],
                                 rhs=dum[:, :], start=True, stop=True)

            # --- input DMAs: warmup consumes x0[:,k] + first half of w1sb[k]
            # per k-step; emit in that order, with a tiny first chunk so the
            # first matmul fires as early as possible.
            nb0 = blocks[0]
            w1sb = []
            for k in range(KD):
                t = wpool.tile([128, F], f16, tag=f"w1_{k}", name=f"w1_{k}")
                w1sb.append(t)
            x0 = xpool.tile([128, KD, 512], f16, tag="x")
            nc.sync.dma_start(out=w1sb[0][:, 0:128], in_=W1[0:128, 0:128])
            nc.sync.dma_start(out=x0[:, 0:1, :nb0], in_=slab(xT, 0, nb0)[:, 0:1, :])
            nc.sync.dma_start(out=w1sb[0][:, 128:F // 2], in_=W1[0:128, 128:F // 2])
            for k in range(1, KD):
                nc.sync.dma_start(out=w1sb[k][:, 0:F // 2],
                                  in_=W1[k * 128:(k + 1) * 128, 0:F // 2])
                nc.sync.dma_start(out=x0[:, k:k + 1, :nb0],
                                  in_=slab(xT, 0, nb0)[:, k:k + 1, :])
            for k in range(KD):
                nc.sync.dma_start(out=w1sb[k][:, F // 2:F],
                                  in_=W1[k * 128:(k + 1) * 128, F // 2:F])

            wb0 = wbpool.tile([128, 512], f32, tag="wb")
            nc.sync.dma_start(out=wb0[:, :nb0], in_=bcast(wt, 0, nb0))

            w3sb = wpool.tile([128, KD, F], f16, tag="w3")
            nc.sync.dma_start(
                out=w3sb[:, :, 0:F // 2],
                in_=W3[:, 0:F // 2].rearrange("(k p) n -> p k n", p=128))
            nc.sync.dma_start(
                out=w3sb[:, :, F // 2:F],
                in_=W3[:, F // 2:F].rearrange("(k p) n -> p k n", p=128))
            w2sb = wpool.tile([128, KF, D], f16, tag="w2")
            nc.sync.dma_start(
                out=w2sb[:, :, 0:D // 2],
                in_=W2[:, 0:D // 2].rearrange("(k p) n -> p k n", p=128))
            nc.sync.dma_start(
                out=w2sb[:, :, D // 2:D],
                in_=W2[:, D // 2:D].rearrange("(k p) n -> p k n", p=128))

            # x/wt for the remaining main blocks
            xts, wbs = {0: x0}, {0: wb0}
            c0 = blocks[0]
            for b in range(1, len(blocks)):
                nb = blocks[b]
                xsb = xpool.tile([128, KD, 512], f16, tag="x")
                nc.sync.dma_start(out=xsb[:, :, :nb], in_=slab(xT, c0, nb))
                wb = wbpool.tile([128, 512], f32, tag="wb")
                nc.sync.dma_start(out=wb[:, :nb], in_=bcast(wt, c0, nb))
                xts[b], wbs[b] = xsb, wb
                c0 += nb

            # tail x / combine weights; tail weight windows stream in-loop
            if R:
                xt = tspool.tile([128, KD, R], f16, tag="xt")
                nc.sync.dma_start(out=xt, in_=slab(xTt, 0, R))
                wbt = wbpool.tile([128, 512], f32, tag="wbt")
                nc.sync.dma_start(out=wbt[:, :R], in_=bcast(wtt, 0, R))

            def tail_w13_chunk(Wsrc, f, tag):
                t = twpool.tile([128, KD, 128], f16, tag=tag, name=f"{tag}_{f}")
                nc.sync.dma_start(
                    out=t,
                    in_=Wsrc[:, f * 128:(f + 1) * 128].rearrange(
                        "(k p) n -> p k n", p=128))
                return t

            def tail_w2_chunk(dd):
                t = w2tpool.tile([128, KF, 128], f16, tag="w2t", name=f"w2t_{dd}")
                nc.sync.dma_start(
                    out=t,
                    in_=W2t[:, dd * 128:(dd + 1) * 128].rearrange(
                        "(k p) n -> p k n", p=128))
                return t

            c0 = 0
            for b, nb in enumerate(blocks):
                xsb, wb = xts[b], wbs[b]
                tail = R and b == last

                # Pass 1: h1 = W1^T x, s = silu(h1)
                sts = [None] * KF
                if b == 0:
                    # k-outer over the first 7 f-tiles with 7 PSUM banks:
                    # each landed W1 k-tile chunk immediately feeds 7
                    # matmuls, so warmup runs under the W1/x DMA stream.
                    pss = [
                        psA.tile([128, 512], f32, tag="ps1", name=f"ps1w{f}")
                        for f in range(3)
                    ] + [
                        psB.tile([128, 512], f32, tag="ps3", name=f"ps3w{f}")
                        for f in range(2)
                    ] + [
                        psY.tile([128, 512], f32, tag="psy", name=f"psyw{f}")
                        for f in range(2)
                    ] + [
                        psT.tile([128, 512], f32, tag="pst", name="pstw"),
                    ]
                    for k in range(KD):
                        for f in range(8):
                            fs = slice(f * 128, (f + 1) * 128)
                            nc.tensor.matmul(
                                pss[f][:, :nb], lhsT=w1sb[k][:, fs],
                                rhs=xsb[:, k, :nb],
                                start=(k == 0), stop=(k == KD - 1),
                            )
                    for f in range(8):
                        s = spool.tile([128, 512], f16, tag=f"s{f}")
                        nc.scalar.activation(
                            s[:, :nb], pss[f][:, :nb],
                            mybir.ActivationFunctionType.Silu,
                        )
                        sts[f] = s
                    # second k-outer group (f=8..11): consumes the W1
                    # second-half slabs k-progressively, matching DMA
                    # arrival order instead of stalling on the last slab
                    pss2 = [
                        psA.tile([128, 512], f32, tag="ps1", name=f"ps1v{f}")
                        for f in range(2)
                    ] + [
                        psB.tile([128, 512], f32, tag="ps3", name=f"ps3v{f}")
                        for f in range(2)
                    ]
                    for k in range(KD):
                        for i, f in enumerate(range(8, 12)):
                            fs = slice(f * 128, (f + 1) * 128)
                            nc.tensor.matmul(
                                pss2[i][:, :nb], lhsT=w1sb[k][:, fs],
                                rhs=xsb[:, k, :nb],
                                start=(k == 0), stop=(k == KD - 1),
                            )
                    for i, f in enumerate(range(8, 12)):
                        s = spool.tile([128, 512], f16, tag=f"s{f}")
                        nc.scalar.activation(
                            s[:, :nb], pss2[i][:, :nb],
                            mybir.ActivationFunctionType.Silu,
                        )
                        sts[f] = s
                if tail:
                    w1tw = {ff: tail_w13_chunk(W1t, ff, "w1t")
                            for ff in range(4)}
                    w3tw = {}
                for f in range(12 if b == 0 else 0, KF):
                    fs = slice(f * 128, (f + 1) * 128)
                    ps1 = psA.tile([128, 512], f32, tag="ps1")
                    if tail:
                        ps1t = psT.tile([128, 512], f32, tag="pst")
                        w1tf = w1tw[f]
                    for k in range(KD):
                        nc.tensor.matmul(
                            ps1[:, :nb], lhsT=w1sb[k][:, fs], rhs=xsb[:, k, :nb],
                            start=(k == 0), stop=(k == KD - 1),
                        )
                        if tail:
                            nc.tensor.matmul(
                                ps1t[:, :R], lhsT=w1tf[:, k, :], rhs=xt[:, k, :],
                                start=(k == 0), stop=(k == KD - 1),
                            )
                    if tail and f + 4 < KF:
                        w1tw[f + 4] = tail_w13_chunk(W1t, f + 4, "w1t")
                    if tail and f >= KF - 4:
                        w3tw[f - (KF - 4)] = tail_w13_chunk(
                            W3t, f - (KF - 4), "w3t")
                    s = spool.tile([128, 512], f16, tag=f"s{f}")
                    nc.scalar.activation(
                        s[:, :nb], ps1[:, :nb], mybir.ActivationFunctionType.Silu
                    )
                    sts[f] = s
                    if tail:
                        st = tspool.tile([128, R], f16, tag=f"st{f}")
                        nc.scalar.activation(
                            st[:, :], ps1t[:, :R],
                            mybir.ActivationFunctionType.Silu,
                        )
                        sts[f] = (s, st)

                # Pass 2: h3 = W3^T x, g = s * h3
                gts = []
                if tail:
                    w2tw = {}
                for f in range(KF):
                    fs = slice(f * 128, (f + 1) * 128)
                    ps3 = psB.tile([128, 512], f32, tag="ps3")
                    if tail:
                        ps3t = psT.tile([128, 512], f32, tag="pst")
                        w3tf = w3tw[f]
                    for k in range(KD):
                        nc.tensor.matmul(
                            ps3[:, :nb], lhsT=w3sb[:, k, fs], rhs=xsb[:, k, :nb],
                            start=(k == 0), stop=(k == KD - 1),
                        )
                        if tail:
                            nc.tensor.matmul(
                                ps3t[:, :R], lhsT=w3tf[:, k, :], rhs=xt[:, k, :],
                                start=(k == 0), stop=(k == KD - 1),
                            )
                    if tail and f + 4 < KF:
                        w3tw[f + 4] = tail_w13_chunk(W3t, f + 4, "w3t")
                    if tail and f >= KF - 4:
                        w2tw[f - (KF - 4)] = tail_w2_chunk(f - (KF - 4))
                    g = gpool.tile([128, 512], f16, tag=f"g{f}")
                    if tail:
                        s, st = sts[f]
                        nc.vector.tensor_mul(g[:, :nb], s[:, :nb], ps3[:, :nb])
                        gt = tspool.tile([128, R], f16, tag=f"gt{f}")
                        nc.vector.tensor_mul(gt[:, :], st[:, :], ps3t[:, :R])
                        gts.append((g, gt))
                    else:
                        nc.vector.tensor_mul(g[:, :nb], sts[f][:, :nb], ps3[:, :nb])
                        gts.append(g)

                # Pass 3: y^T = (W2^T g) * w
                ysb = ypool.tile([128, KD, 512], f16, tag="y")
                for dd in range(KD):
                    ds_ = slice(dd * 128, (dd + 1) * 128)
                    psy = psY.tile([128, 512], f32, tag="psy")
                    if tail:
                        psyt = psT.tile([128, 512], f32, tag="pst")
                        w2td = w2tw[dd]
                    # On the very last tile, the final PSUM-stop semaphore is
                    # gated behind the PE's ~2us exit drain; order the main
                    # matmuls/mul/DMA first and the tiny tail matmuls last so
                    # only the 87ns tail mul + 11KB DMA pay that latency.
                    split_last = tail and b == last and dd == KD - 1
                    for f in range(KF):
                        if tail:
                            g, gt = gts[f]
                        else:
                            g = gts[f]
                        nc.tensor.matmul(
                            psy[:, :nb], lhsT=w2sb[:, f, ds_], rhs=g[:, :nb],
                            start=(f == 0), stop=(f == KF - 1),
                        )
                        if tail and not split_last:
                            nc.tensor.matmul(
                                psyt[:, :R], lhsT=w2td[:, f, :], rhs=gt[:, :],
                                start=(f == 0), stop=(f == KF - 1),
                            )
                    if tail and dd + 4 < KD:
                        w2tw[dd + 4] = tail_w2_chunk(dd + 4)
                    if split_last:
                        nc.vector.tensor_mul(ysb[:, dd, :nb], psy[:, :nb],
                                             wb[:, :nb])
                        nc.sync.dma_start(
                            out=slab(yT, c0, nb)[:, dd:dd + 1, :],
                            in_=ysb[:, dd:dd + 1, :nb],
                        )
                        for f in range(KF):
                            _, gt = gts[f]
                            nc.tensor.matmul(
                                psyt[:, :R], lhsT=w2td[:, f, :], rhs=gt[:, :],
                                start=(f == 0), stop=(f == KF - 1),
                            )
                        ytl = tspool.tile([128, R], f16, tag="yt")
                        nc.vector.tensor_mul(ytl[:, :], psyt[:, :R], wbt[:, :R])
                        nc.sync.dma_start(
                            out=yTt[dd * 128:(dd + 1) * 128, :], in_=ytl[:, :],
                        )
                        c0 += nb
                        continue
                    if tail:
                        ytl = tspool.tile([128, R], f16, tag="yt")
                        nc.vector.tensor_mul(ytl[:, :], psyt[:, :R], wbt[:, :R])
                        nc.sync.dma_start(
                            out=yTt[dd * 128:(dd + 1) * 128, :], in_=ytl[:, :],
                        )
                    nc.vector.tensor_mul(ysb[:, dd, :nb], psy[:, :nb], wb[:, :nb])
                    if b == last:
                        # last block: per-tile output DMAs so the kernel-tail
                        # drain only waits on a tiny final transfer
                        nc.sync.dma_start(
                            out=slab(yT, c0, nb)[:, dd:dd + 1, :],
                            in_=ysb[:, dd:dd + 1, :nb],
                        )
                    elif dd == KD // 2 - 1:
                        nc.sync.dma_start(
                            out=slab(yT, c0, nb)[:, 0:KD // 2, :],
                            in_=ysb[:, 0:KD // 2, :nb],
                        )
                if b != last:
                    nc.sync.dma_start(
                        out=slab(yT, c0, nb)[:, KD // 2:KD, :],
                        in_=ysb[:, KD // 2:KD, :nb],
                    )
                c0 += nb
    nc.finalize()
    return nc


def _route(x, Wg):
    """Top-2 softmax routing in float64 (top-2/top-3 gaps are >>f32 eps, so
    this matches the f32 reference selection exactly)."""
    logits = x.astype(np.float64) @ Wg.astype(np.float64)
    logits -= logits.max(axis=-1, keepdims=True)
    g = np.exp(logits)
    g /= g.sum(axis=-1, keepdims=True)
    top_i = np.argpartition(-g, TOPK - 1, axis=-1)[:, :TOPK]      # [T, 2]
    tg = np.take_along_axis(g, top_i, axis=-1)
    tg = tg / tg.sum(axis=-1, keepdims=True)
    return top_i, tg


def _plan(counts):
    """Choose tail width R and split overloaded experts' overflow into at
    most N_CORES parts of <= R tokens each."""
    excess = {e: c - MAIN for e, c in enumerate(counts) if c > MAIN}
    if not excess:
        return 0, []
    R = None
    for r in range(max(1, -(-sum(excess.values()) // N_CORES)), 513):
        if sum(-(-v // r) for v in excess.values()) <= N_CORES:
            R = r
            break
    if R is None:
        raise RuntimeError(f"infeasible tail packing: {excess}")
    parts = []
    for e, v in excess.items():
        n = -(-v // R)
        sizes = [v // n + (1 if i < v % n else 0) for i in range(n)]
        off = MAIN
        for sz in sizes:
            parts.append((e, off, sz))
            off += sz
    return R, parts


def run(inputs, trace=False, trace_cores=None):
    hidden_states = np.asarray(inputs["hidden_states"], dtype=np.float32)
    Wg = np.asarray(inputs["Wg"], dtype=np.float32)
    W1 = np.asarray(inputs["W1"], dtype=np.float32)
    W3 = np.asarray(inputs["W3"], dtype=np.float32)
    W2 = np.asarray(inputs["W2"], dtype=np.float32)

    x = hidden_states.reshape(-1, D)                              # [T, D]
    T = x.shape[0]
    top_i, tg = _route(x, Wg)

    idx = []
    wts = []
    for e in range(E):
        sel = top_i == e                                          # [T, 2]
        rows = np.where(sel.any(axis=-1))[0]
        idx.append(rows)
        wts.append(np.where(sel[rows, 0], tg[rows, 0], tg[rows, 1]))
    counts = [len(r) for r in idx]

    if max(counts) > MAIN:
        C = MAIN
        R, parts = _plan(counts)
    else:
        C = max(max(counts), 1)
        R, parts = 0, []

    key = (C, R)
    if key not in _nc_cache:
        _nc_cache[key] = _build_nc(C, R)
    nc = _nc_cache[key]

    W1h = [W1[e].astype(np.float16) for e in range(E)]
    W3h = [W3[e].astype(np.float16) for e in range(E)]
    W2h = [W2[e].astype(np.float16) for e in range(E)]

    in_maps = []
    for e in range(E):
        rows = idx[e][:C]
        c = len(rows)
        xTe = np.zeros((D, C), np.float16)
        xTe[:, :c] = x[rows].T
        wte = np.zeros((1, C), np.float32)
        wte[0, :c] = wts[e][:c]
        m = {"xT": xTe, "wt": wte, "W1": W1h[e], "W3": W3h[e], "W2": W2h[e]}
        if R:
            if e < len(parts):
                te, off, sz = parts[e]
                trows = idx[te][off:off + sz]
                xTte = np.zeros((D, R), np.float16)
                xTte[:, :sz] = x[trows].T
                wtte = np.zeros((1, R), np.float32)
                wtte[0, :sz] = wts[te][off:off + sz]
                m.update({"xTt": xTte, "wtt": wtte, "W1t": W1h[te],
                          "W3t": W3h[te], "W2t": W2h[te]})
            else:
                m.update({"xTt": np.zeros((D, R), np.float16),
                          "wtt": np.zeros((1, R), np.float32),
                          "W1t": W1h[e], "W3t": W3h[e], "W2t": W2h[e]})
        in_maps.append(m)

    kwargs = {}
    if trace:
        kwargs["trace"] = True
        kwargs["trace_cores"] = trace_cores or list(range(N_CORES))
    res = run_bass_kernel_spmd(nc, in_maps, list(range(N_CORES)), **kwargs)

    out = np.zeros((T, D), np.float32)
    for e in range(E):
        c = min(len(idx[e]), C)
        if c:
            out[idx[e][:c]] += res.results[e]["yT"][:, :c].T.astype(np.float32)
        if R and e < len(parts):
            te, off, sz = parts[e]
            trows = idx[te][off:off + sz]
            out[trows] += res.results[e]["yTt"][:, :sz].T.astype(np.float32)
    return out.reshape(B, S, D), res


def kernel(**inputs):
    out, _ = run(inputs, trace=False)
    return out


# revision 20
# speedup vs baseline: 1.0114x; 1.0114x over previous
"""MoE feed-forward (B=4,S=2048,D=1024,F=2048,E=8,top-2) on 8 trn2 NeuronCores.

Strategy (expert-parallel + balanced tail):
 - Host computes the top-2 softmax routing and dispatches tokens.
 - Core e's "main" blocks hold up to 2048 tokens of expert e (4 blocks of
   512, transposed to [D, 2048] fp16, zero padded).
 - Tokens beyond 2048 of overloaded experts are split into parts of <= R
   and placed in a per-core "tail" block of width R whose weight set is an
   independent kernel input -- so each core processes 2048+R columns
   instead of C_max, balancing the matmul stream across cores.
 - Device per core: h1 = W1^T x, s = silu(h1), h3 = W3^T x, g = s*h3,
   y^T = (W2^T g) * w, streamed over column blocks; all matmuls
   fp16 x fp16 -> fp32 PSUM.  Tail matmuls ride along the last main
   block's (f,k) loops; tail weights stream through small SBUF windows
   with DMA emission interleaved into the loops (just-in-time, so the
   DMA queue never blocks on window reuse).
 - A memset + 8 garbage matmuls at t=0 warm the PE HAM clock gate while
   the first real DMAs land, so real matmuls start at 2.4 GHz.
"""

import numpy as np

import concourse.bass as bass
import concourse.tile as tile
from concourse import bacc, mybir
from concourse.bass_utils import run_bass_kernel_spmd

B, S, D, F, E, TOPK = 4, 2048, 1024, 2048, 8, 2
N_CORES = 8
KD = D // 128   # 8 contraction tiles for D
KF = F // 128   # 16 contraction tiles for F
MAIN = 2048     # main capacity per core

_nc_cache = {}


def _blocks_for(C):
    """Split C columns into blocks of <=512, all >=~251 where possible."""
    if C <= 512:
        return [C]
    n = -(-C // 512)
    r = C - 512 * (n - 2)
    a = (r + 1) // 2
    return [512] * (n - 2) + [a, r - a]


def _build_nc(C, R):
    """Per-core Bass program: main capacity C (blocks of <=512) plus an
    optional tail block of width R with its own weight set."""
    f16 = mybir.dt.float16
    f32 = mybir.dt.float32
    blocks = _blocks_for(C)
    last = len(blocks) - 1

    nc = bacc.Bacc(None, target_bir_lowering=False, enable_partition_id=False,
                   monotonic_sem_count=0)
    xT = nc.dram_tensor("xT", [D, C], f16, kind="ExternalInput")
    wt = nc.dram_tensor("wt", [1, C], f32, kind="ExternalInput")
    W1 = nc.dram_tensor("W1", [D, F], f16, kind="ExternalInput")
    W3 = nc.dram_tensor("W3", [D, F], f16, kind="ExternalInput")
    W2 = nc.dram_tensor("W2", [F, D], f16, kind="ExternalInput")
    yT = nc.dram_tensor("yT", [D, C], f16, kind="ExternalOutput")
    if R:
        xTt = nc.dram_tensor("xTt", [D, R], f16, kind="ExternalInput")
        wtt = nc.dram_tensor("wtt", [1, R], f32, kind="ExternalInput")
        W1t = nc.dram_tensor("W1t", [D, F], f16, kind="ExternalInput")
        W3t = nc.dram_tensor("W3t", [D, F], f16, kind="ExternalInput")
        W2t = nc.dram_tensor("W2t", [F, D], f16, kind="ExternalInput")
        yTt = nc.dram_tensor("yTt", [D, R], f16, kind="ExternalOutput")

    # [D, nb] slab viewed as [128, KD, nb] (partition-major tiles)
    def slab(t, c0, nb):
        return t[:, c0:c0 + nb].rearrange("(k p) n -> p k n", p=128)

    def bcast(t, c0, nb):
        return bass.AP(tensor=t.ap().tensor, offset=c0, ap=[[0, 128], [1, nb]])

    with tile.TileContext(nc) as tc:
        with (
            tc.tile_pool(name="wpool", bufs=1) as wpool,
            tc.tile_pool(name="xpool", bufs=2) as xpool,
            tc.tile_pool(name="gpool", bufs=1) as gpool,
            tc.tile_pool(name="spool", bufs=1) as spool,
            tc.tile_pool(name="ypool", bufs=1) as ypool,
            tc.tile_pool(name="wbpool", bufs=2) as wbpool,
            tc.tile_pool(name="twpool", bufs=5) as twpool,
            tc.tile_pool(name="w2tpool", bufs=5) as w2tpool,
            tc.tile_pool(name="tspool", bufs=1) as tspool,
            tc.tile_pool(name="psA", bufs=3, space="PSUM") as psA,
            tc.tile_pool(name="psB", bufs=2, space="PSUM") as psB,
            tc.tile_pool(name="psY", bufs=2, space="PSUM") as psY,
            tc.tile_pool(name="psT", bufs=1, space="PSUM") as psT,
        ):
            # --- HAM warmup: PE busy on garbage while first DMAs land ---
            dum = spool.tile([128, 256], f16, tag="dum")
            nc.vector.memset(dum[:, :], 0.0)
            psd = psA.tile([128, 512], f32, tag="ps1", name="psdum")
            for _ in range(16):
                nc.tensor.matmul(psd[:, 0:256], lhsT=dum[:, 0:128],
                                 rhs=dum[:, :], start=True, stop=True)

            # --- input DMAs: warmup consumes x0[:,k] + first half of w1sb[k]
            # per k-step; emit in that order, with a tiny first chunk so the
            # first matmul fires as early as possible.
            nb0 = blocks[0]
            w1sb = []
            for k in range(KD):
                t = wpool.tile([128, F], f16, tag=f"w1_{k}", name=f"w1_{k}")
                w1sb.append(t)
            x0 = xpool.tile([128, KD, 512], f16, tag="x")
            nc.sync.dma_start(out=w1sb[0][:, 0:128], in_=W1[0:128, 0:128])
            nc.sync.dma_start(out=x0[:, 0:1, :nb0], in_=slab(xT, 0, nb0)[:, 0:1, :])
            nc.sync.dma_start(out=w1sb[0][:, 128:F // 2], in_=W1[0:128, 128:F // 2])
            for k in range(1, KD):
                nc.sync.dma_start(out=w1sb[k][:, 0:F // 2],
                                  in_=W1[k * 128:(k + 1) * 128, 0:F // 2])
                nc.sync.dma_start(out=x0[:, k:k + 1, :nb0],
                                  in_=slab(xT, 0, nb0)[:, k:k + 1, :])
            for k in range(KD):
                nc.sync.dma_start(out=w1sb[k][:, F // 2:F],
                                  in_=W1[k * 128:(k + 1) * 128, F // 2:F])

            wb0 = wbpool.tile([128, 512], f32, tag="wb")
            nc.sync.dma_start(out=wb0[:, :nb0], in_=bcast(wt, 0, nb0))

            w3sb = wpool.tile([128, KD, F], f16, tag="w3")
            nc.sync.dma_start(
                out=w3sb[:, :, 0:F // 2],
                in_=W3[:, 0:F // 2].rearrange("(k p) n -> p k n", p=128))
            nc.sync.dma_start(
                out=w3sb[:, :, F // 2:F],
                in_=W3[:, F // 2:F].rearrange("(k p) n -> p k n", p=128))
            w2sb = wpool.tile([128, KF, D], f16, tag="w2")
            nc.sync.dma_start(
                out=w2sb[:, :, 0:D // 2],
                in_=W2[:, 0:D // 2].rearrange("(k p) n -> p k n", p=128))
            nc.sync.dma_start(
                out=w2sb[:, :, D // 2:D],
                in_=W2[:, D // 2:D].rearrange("(k p) n -> p k n", p=128))

            # x/wt for the remaining main blocks
            xts, wbs = {0: x0}, {0: wb0}
            c0 = blocks[0]
            for b in range(1, len(blocks)):
                nb = blocks[b]
                xsb = xpool.tile([128, KD, 512], f16, tag="x")
                nc.sync.dma_start(out=xsb[:, :, :nb], in_=slab(xT, c0, nb))
                wb = wbpool.tile([128, 512], f32, tag="wb")
                nc.sync.dma_start(out=wb[:, :nb], in_=bcast(wt, c0, nb))
                xts[b], wbs[b] = xsb, wb
                c0 += nb

            # tail x / combine weights; tail weight windows stream in-loop
            if R:
                xt = tspool.tile([128, KD, R], f16, tag="xt")
                nc.sync.dma_start(out=xt, in_=slab(xTt, 0, R))
                wbt = wbpool.tile([128, 512], f32, tag="wbt")
                nc.sync.dma_start(out=wbt[:, :R], in_=bcast(wtt, 0, R))

            def tail_w13_chunk(Wsrc, f, tag):
                t = twpool.tile([128, KD, 128], f16, tag=tag, name=f"{tag}_{f}")
                nc.sync.dma_start(
                    out=t,
                    in_=Wsrc[:, f * 128:(f + 1) * 128].rearrange(
                        "(k p) n -> p k n", p=128))
                return t

            def tail_w2_chunk(dd):
                t = w2tpool.tile([128, KF, 128], f16, tag="w2t", name=f"w2t_{dd}")
                nc.sync.dma_start(
                    out=t,
                    in_=W2t[:, dd * 128:(dd + 1) * 128].rearrange(
                        "(k p) n -> p k n", p=128))
                return t

            c0 = 0
            w1tw = {}
            for b, nb in enumerate(blocks):
                xsb, wb = xts[b], wbs[b]
                tail = R and b == last

                # Pass 1: h1 = W1^T x, s = silu(h1)
                sts = [None] * KF
                if b == 0:
                    # k-outer over the first 7 f-tiles with 7 PSUM banks:
                    # each landed W1 k-tile chunk immediately feeds 7
                    # matmuls, so warmup runs under the W1/x DMA stream.
                    pss = [
                        psA.tile([128, 512], f32, tag="ps1", name=f"ps1w{f}")
                        for f in range(3)
                    ] + [
                        psB.tile([128, 512], f32, tag="ps3", name=f"ps3w{f}")
                        for f in range(2)
                    ] + [
                        psY.tile([128, 512], f32, tag="psy", name=f"psyw{f}")
                        for f in range(2)
                    ] + [
                        psT.tile([128, 512], f32, tag="pst", name="pstw"),
                    ]
                    for k in range(KD):
                        for f in range(8):
                            fs = slice(f * 128, (f + 1) * 128)
                            nc.tensor.matmul(
                                pss[f][:, :nb], lhsT=w1sb[k][:, fs],
                                rhs=xsb[:, k, :nb],
                                start=(k == 0), stop=(k == KD - 1),
                            )
                    for f in range(8):
                        s = spool.tile([128, 512], f16, tag=f"s{f}")
                        nc.scalar.activation(
                            s[:, :nb], pss[f][:, :nb],
                            mybir.ActivationFunctionType.Silu,
                        )
                        sts[f] = s
                    # second k-outer group (f=8..11): consumes the W1
                    # second-half slabs k-progressively, matching DMA
                    # arrival order instead of stalling on the last slab
                    pss2 = [
                        psA.tile([128, 512], f32, tag="ps1", name=f"ps1v{f}")
                        for f in range(2)
                    ] + [
                        psB.tile([128, 512], f32, tag="ps3", name=f"ps3v{f}")
                        for f in range(2)
                    ]
                    for k in range(KD):
                        for i, f in enumerate(range(8, 12)):
                            fs = slice(f * 128, (f + 1) * 128)
                            nc.tensor.matmul(
                                pss2[i][:, :nb], lhsT=w1sb[k][:, fs],
                                rhs=xsb[:, k, :nb],
                                start=(k == 0), stop=(k == KD - 1),
                            )
                    for i, f in enumerate(range(8, 12)):
                        s = spool.tile([128, 512], f16, tag=f"s{f}")
                        nc.scalar.activation(
                            s[:, :nb], pss2[i][:, :nb],
                            mybir.ActivationFunctionType.Silu,
                        )
                        sts[f] = s
                if tail:
                    if not w1tw:
                        w1tw = {ff: tail_w13_chunk(W1t, ff, "w1t")
                                for ff in range(4)}
                    w3tw = {}
                for f in range(12 if b == 0 else 0, KF):
                    fs = slice(f * 128, (f + 1) * 128)
                    ps1 = psA.tile([128, 512], f32, tag="ps1")
                    if tail:
                        ps1t = psT.tile([128, 512], f32, tag="pst")
                        w1tf = w1tw[f]
                    for k in range(KD):
                        nc.tensor.matmul(
                            ps1[:, :nb], lhsT=w1sb[k][:, fs], rhs=xsb[:, k, :nb],
                            start=(k == 0), stop=(k == KD - 1),
                        )
                        if tail:
                            nc.tensor.matmul(
                                ps1t[:, :R], lhsT=w1tf[:, k, :], rhs=xt[:, k, :],
                                start=(k == 0), stop=(k == KD - 1),
                            )
                    if tail and f + 4 < KF:
                        w1tw[f + 4] = tail_w13_chunk(W1t, f + 4, "w1t")
                    if tail and KF - 8 <= f < KF - 4:
                        w3tw[f - (KF - 8)] = tail_w13_chunk(
                            W3t, f - (KF - 8), "w3t")
                    s = spool.tile([128, 512], f16, tag=f"s{f}")
                    nc.scalar.activation(
                        s[:, :nb], ps1[:, :nb], mybir.ActivationFunctionType.Silu
                    )
                    sts[f] = s
                    if tail:
                        st = tspool.tile([128, R], f16, tag=f"st{f}")
                        nc.scalar.activation(
                            st[:, :], ps1t[:, :R],
                            mybir.ActivationFunctionType.Silu,
                        )
                        sts[f] = (s, st)

                # Pass 2: h3 = W3^T x, g = s * h3
                gts = []
                if tail:
                    w2tw = {}
                for f in range(KF):
                    fs = slice(f * 128, (f + 1) * 128)
                    ps3 = psB.tile([128, 512], f32, tag="ps3")
                    if tail:
                        ps3t = psT.tile([128, 512], f32, tag="pst")
                        w3tf = w3tw[f]
                    for k in range(KD):
                        nc.tensor.matmul(
                            ps3[:, :nb], lhsT=w3sb[:, k, fs], rhs=xsb[:, k, :nb],
                            start=(k == 0), stop=(k == KD - 1),
                        )
                        if tail:
                            nc.tensor.matmul(
                                ps3t[:, :R], lhsT=w3tf[:, k, :], rhs=xt[:, k, :],
                                start=(k == 0), stop=(k == KD - 1),
                            )
                    if tail and f + 4 < KF:
                        w3tw[f + 4] = tail_w13_chunk(W3t, f + 4, "w3t")
                    if tail and KF - 8 <= f < KF - 4:
                        w2tw[f - (KF - 8)] = tail_w2_chunk(f - (KF - 8))
                    g = gpool.tile([128, 512], f16, tag=f"g{f}")
                    if tail:
                        s, st = sts[f]
                        nc.vector.tensor_mul(g[:, :nb], s[:, :nb], ps3[:, :nb])
                        gt = tspool.tile([128, R], f16, tag=f"gt{f}")
                        nc.vector.tensor_mul(gt[:, :], st[:, :], ps3t[:, :R])
                        gts.append((g, gt))
                    else:
                        nc.vector.tensor_mul(g[:, :nb], sts[f][:, :nb], ps3[:, :nb])
                        gts.append(g)

                # Pass 3: y^T = (W2^T g) * w
                if R and b == last - 1:
                    # seed the tail block's W1 windows a full block-phase
                    # early: ~28us of delivery margin instead of ~1us
                    for ff in range(4):
                        w1tw[ff] = tail_w13_chunk(W1t, ff, "w1t")
                ysb = ypool.tile([128, KD, 512], f16, tag="y")
                for dd in range(KD):
                    ds_ = slice(dd * 128, (dd + 1) * 128)
                    psy = psY.tile([128, 512], f32, tag="psy")
                    if tail:
                        psyt = psT.tile([128, 512], f32, tag="pst")
                        w2td = w2tw[dd]
                    # On the very last tile, the final PSUM-stop semaphore is
                    # gated behind the PE's ~2us exit drain; order the main
                    # matmuls/mul/DMA first and the tiny tail matmuls last so
                    # only the 87ns tail mul + 11KB DMA pay that latency.
                    split_last = tail and b == last and dd == KD - 1
                    for f in range(KF):
                        if tail:
                            g, gt = gts[f]
                        else:
                            g = gts[f]
                        nc.tensor.matmul(
                            psy[:, :nb], lhsT=w2sb[:, f, ds_], rhs=g[:, :nb],
                            start=(f == 0), stop=(f == KF - 1),
                        )
                        if tail and not split_last:
                            nc.tensor.matmul(
                                psyt[:, :R], lhsT=w2td[:, f, :], rhs=gt[:, :],
                                start=(f == 0), stop=(f == KF - 1),
                            )
                    if tail and dd + 4 < KD:
                        w2tw[dd + 4] = tail_w2_chunk(dd + 4)
                    if split_last:
                        nc.vector.tensor_mul(ysb[:, dd, :nb], psy[:, :nb],
                                             wb[:, :nb])
                        nc.sync.dma_start(
                            out=slab(yT, c0, nb)[:, dd:dd + 1, :],
                            in_=ysb[:, dd:dd + 1, :nb],
                        )
                        for f in range(KF):
                            _, gt = gts[f]
                            nc.tensor.matmul(
                                psyt[:, :R], lhsT=w2td[:, f, :], rhs=gt[:, :],
                                start=(f == 0), stop=(f == KF - 1),
                            )
                        ytl = tspool.tile([128, R], f16, tag="yt")
                        nc.vector.tensor_mul(ytl[:, :], psyt[:, :R], wbt[:, :R])
                        nc.sync.dma_start(
                            out=yTt[dd * 128:(dd + 1) * 128, :], in_=ytl[:, :],
                        )
                        c0 += nb
                        continue
                    if tail:
                        ytl = tspool.tile([128, R], f16, tag="yt")
                        nc.vector.tensor_mul(ytl[:, :], psyt[:, :R], wbt[:, :R])
                        nc.sync.dma_start(
                            out=yTt[dd * 128:(dd + 1) * 128, :], in_=ytl[:, :],
                        )
                    nc.vector.tensor_mul(ysb[:, dd, :nb], psy[:, :nb], wb[:, :nb])
                    if b == last:
                        # last block: per-tile output DMAs so the kernel-tail
                        # drain only waits on a tiny final transfer
                        nc.sync.dma_start(
                            out=slab(yT, c0, nb)[:, dd:dd + 1, :],
                            in_=ysb[:, dd:dd + 1, :nb],
                        )
                    elif dd == KD // 2 - 1:
                        nc.sync.dma_start(
                            out=slab(yT, c0, nb)[:, 0:KD // 2, :],
                            in_=ysb[:, 0:KD // 2, :nb],
                        )
                if b != last:
                    nc.sync.dma_start(
                        out=slab(yT, c0, nb)[:, KD // 2:KD, :],
                        in_=ysb[:, KD // 2:KD, :nb],
                    )
                c0 += nb
    nc.finalize()
    return nc


def _route(x, Wg):
    """Top-2 softmax routing in float64 (top-2/top-3 gaps are >>f32 eps, so
    this matches the f32 reference selection exactly)."""
    logits = x.astype(np.float64) @ Wg.astype(np.float64)
    logits -= logits.max(axis=-1, keepdims=True)
    g = np.exp(logits)
    g /= g.sum(axis=-1, keepdims=True)
    top_i = np.argpartition(-g, TOPK - 1, axis=-1)[:, :TOPK]      # [T, 2]
    tg = np.take_along_axis(g, top_i, axis=-1)
    tg = tg / tg.sum(axis=-1, keepdims=True)
    return top_i, tg


def _plan(counts):
    """Choose tail width R and split overloaded experts' overflow into at
    most N_CORES parts of <= R tokens each."""
    excess = {e: c - MAIN for e, c in enumerate(counts) if c > MAIN}
    if not excess:
        return 0, []
    R = None
    for r in range(max(1, -(-sum(excess.values()) // N_CORES)), 513):
        if sum(-(-v // r) for v in excess.values()) <= N_CORES:
            R = r
            break
    if R is None:
        raise RuntimeError(f"infeasible tail packing: {excess}")
    parts = []
    for e, v in excess.items():
        n = -(-v // R)
        sizes = [v // n + (1 if i < v % n else 0) for i in range(n)]
        off = MAIN
        for sz in sizes:
            parts.append((e, off, sz))
            off += sz
    return R, parts


def run(inputs, trace=False, trace_cores=None):
    hidden_states = np.asarray(inputs["hidden_states"], dtype=np.float32)
    Wg = np.asarray(inputs["Wg"], dtype=np.float32)
    W1 = np.asarray(inputs["W1"], dtype=np.float32)
    W3 = np.asarray(inputs["W3"], dtype=np.float32)
    W2 = np.asarray(inputs["W2"], dtype=np.float32)

    x = hidden_states.reshape(-1, D)                              # [T, D]
    T = x.shape[0]
    top_i, tg = _route(x, Wg)

    idx = []
    wts = []
    for e in range(E):
        sel = top_i == e                                          # [T, 2]
        rows = np.where(sel.any(axis=-1))[0]
        idx.append(rows)
        wts.append(np.where(sel[rows, 0], tg[rows, 0], tg[rows, 1]))
    counts = [len(r) for r in idx]

    if max(counts) > MAIN:
        C = MAIN
        R, parts = _plan(counts)
    else:
        C = max(max(counts), 1)
        R, parts = 0, []

    key = (C, R)
    if key not in _nc_cache:
        _nc_cache[key] = _build_nc(C, R)
    nc = _nc_cache[key]

    W1h = [W1[e].astype(np.float16) for e in range(E)]
    W3h = [W3[e].astype(np.float16) for e in range(E)]
    W2h = [W2[e].astype(np.float16) for e in range(E)]

    in_maps = []
    for e in range(E):
        rows = idx[e][:C]
        c = len(rows)
        xTe = np.zeros((D, C), np.float16)
        xTe[:, :c] = x[rows].T
        wte = np.zeros((1, C), np.float32)
        wte[0, :c] = wts[e][:c]
        m = {"xT": xTe, "wt": wte, "W1": W1h[e], "W3": W3h[e], "W2": W2h[e]}
        if R:
            if e < len(parts):
                te, off, sz = parts[e]
                trows = idx[te][off:off + sz]
                xTte = np.zeros((D, R), np.float16)
                xTte[:, :sz] = x[trows].T
                wtte = np.zeros((1, R), np.float32)
                wtte[0, :sz] = wts[te][off:off + sz]
                m.update({"xTt": xTte, "wtt": wtte, "W1t": W1h[te],
                          "W3t": W3h[te], "W2t": W2h[te]})
            else:
                m.update({"xTt": np.zeros((D, R), np.float16),
                          "wtt": np.zeros((1, R), np.float32),
                          "W1t": W1h[e], "W3t": W3h[e], "W2t": W2h[e]})
        in_maps.append(m)

    kwargs = {}
    if trace:
        kwargs["trace"] = True
        kwargs["trace_cores"] = trace_cores or list(range(N_CORES))
    res = run_bass_kernel_spmd(nc, in_maps, list(range(N_CORES)), **kwargs)

    out = np.zeros((T, D), np.float32)
    for e in range(E):
        c = min(len(idx[e]), C)
        if c:
            out[idx[e][:c]] += res.results[e]["yT"][:, :c].T.astype(np.float32)
        if R and e < len(parts):
            te, off, sz = parts[e]
            trows = idx[te][off:off + sz]
            out[trows] += res.results[e]["yTt"][:, :sz].T.astype(np.float32)
    return out.reshape(B, S, D), res


def kernel(**inputs):
    out, _ = run(inputs, trace=False)
    return out
